# revision 13
# baseline (speedup 1.0000x reference)
"""Trainium2 Bass kernel for nn_ConceptLayer (sparsemax + top-8 concept layer).

Computes, per token t (row of h):
    logits = h @ We.T + be                      # [*, 64]
    p      = sparsemax(logits)                  # entmax15() in the reference
    c      = top8_sparsify(p)                   # keep top-8 values in place
    h_out  = c @ Wd.T + bd                      # alpha = 1.0 -> h_out == h_rec

Sharding: pure data parallel over tokens (16*4096 = 65536 tokens) across 8
NeuronCores; tiny weights replicated.

Per-core dataflow (tokens tiled 128 at a time; 8 tiles per DMA chunk; compute
batched over 4-tile groups):
  - PE transposes each [128, 128] h chunk (fp32 has no DMA-transpose path);
    encoder runs with WeT stationary and 512 tokens moving per matmul,
    accumulating logits^T [64, 512] in PSUM; a small PE transpose flips each
    tile's logits back to [128tok, 64].
  - sparsemax via sorted top-16 (nc.vector.max / match_replace / max gives the
    16 largest, descending; measured max support size is 9), cumsum via
    tensor_tensor_scan, support/tau via fused scalar_tensor_tensor ops batched
    across the 4-tile group.
  - top-8 mask is `z >= z_sorted[7]` (bit-exact compare against the 8th
    largest logit -- monotone-equivalent to masking p).
  - decoder: PE-transpose concepts (augmented with a ones column so the
    matmul adds bd via an extra contraction row), 2 matmuls of N=384.
"""

import os
import sys

import numpy as np

for _p in ("/opt/trn_rl_repo", "/root/.axon_site/_ro/trn_rl_repo"):
    if os.path.isdir(_p) and _p not in sys.path:
        sys.path.insert(0, _p)

import concourse.bass as bass
import concourse.bacc as bacc
import concourse.tile as tile
from concourse import mybir
from concourse.bass_utils import run_bass_kernel_spmd
from concourse.masks import make_identity

F32 = mybir.dt.float32
OP = mybir.AluOpType
ACTF = mybir.ActivationFunctionType

N_CORES = 8
B, S, D, M = 16, 4096, 768, 64
TOK = B * S
P = 128
DC = D // P          # 6 d-chunks of 128
GB = 4               # token-tiles per compute batch (512 moving columns)
NEG = -1.0e30


def _broadcast_ap(src: bass.AP, parts: int) -> bass.AP:
    """View a 1-D DRAM tensor as [parts, n] with 0-stride partition dim."""
    return bass.AP(tensor=src.tensor, offset=src.offset, ap=[[0, parts]] + list(src.ap))


def _kernel_body(tc: tile.TileContext, ctx, h, We, be, Wd, bd, hout, cout, tok_per_core):
    nc = tc.nc
    tiles = tok_per_core // P
    G = 8 if tiles % 8 == 0 else GB   # token-tiles per DMA chunk
    chunks = tiles // G
    n_batch = G // GB                 # compute batches per chunk

    consts = ctx.enter_context(tc.tile_pool(name="consts", bufs=1))

    ident = consts.tile([P, P], F32)
    make_identity(nc, ident)

    # rho64[p, g, j] = j+1 (j in 0..15) for the batched support test; ones for
    # the cumsum scan / tau.
    rho_i = consts.tile([P, GB, 16], mybir.dt.int32)
    nc.gpsimd.iota(rho_i, pattern=[[0, GB], [1, 16]], base=1, channel_multiplier=0)
    rho = consts.tile([P, GB, 16], F32)
    nc.vector.tensor_copy(rho, rho_i)
    ones16 = consts.tile([P, 16], F32)
    nc.vector.memset(ones16, 1.0)
    ones4 = ones16[:, 0:GB]

    # be broadcast to [P, M] (added to logits during the PSUM->SBUF copy).
    be_b = consts.tile([P, M], F32)
    nc.gpsimd.dma_start(out=be_b, in_=_broadcast_ap(be, P))

    # Encoder weights: We [M, D] -> WeT chunks [P, DC, M] (WeT[d, c] = We[c, d]).
    # Decoder weights: Wd [D, M] -> WdT_aug [M+1, D]; row M = bd so the ones
    # column of the augmented concepts adds the bias.
    we_sb = consts.tile([M, D], F32)
    nc.sync.dma_start(out=we_sb, in_=We)
    wet = consts.tile([P, DC, M], F32)
    wd_view = Wd.rearrange("(j p) m -> p j m", p=P)
    wd_sb = consts.tile([P, DC, M], F32)
    nc.sync.dma_start(out=wd_sb, in_=wd_view)
    wdt = consts.tile([M + 1, D], F32)
    with tc.tile_pool(name="ps_setup", bufs=2, space="PSUM") as ps_setup:
        for j in range(DC):
            pt = ps_setup.tile([P, M], F32, tag="setup")
            nc.tensor.transpose(pt, we_sb[:, j * P : (j + 1) * P], ident[0:M, 0:M])
            nc.scalar.copy(out=wet[:, j, :], in_=pt)
        for j in range(DC):
            pt = ps_setup.tile([M, P], F32, tag="setup")
            nc.tensor.transpose(pt, wd_sb[:, j, :], ident)
            nc.scalar.copy(out=wdt[0:M, j * P : (j + 1) * P], in_=pt)
    nc.gpsimd.dma_start(out=wdt[M : M + 1, :], in_=_broadcast_ap(bd, 1))

    # Streaming pools.
    h_pool = ctx.enter_context(tc.tile_pool(name="h_in", bufs=3))
    ht_pool = ctx.enter_context(tc.tile_pool(name="ht", bufs=2))
    lgt_pool = ctx.enter_context(tc.tile_pool(name="lgt", bufs=2))
    mid = ctx.enter_context(tc.tile_pool(name="mid", bufs=2))
    grp = ctx.enter_context(tc.tile_pool(name="grp", bufs=2))
    ct_pool = ctx.enter_context(tc.tile_pool(name="ct", bufs=2))
    out_pool = ctx.enter_context(tc.tile_pool(name="out", bufs=2))
    ps_ht = ctx.enter_context(tc.tile_pool(name="ps_ht", bufs=3, space="PSUM"))
    ps_lg = ctx.enter_context(tc.tile_pool(name="ps_lg", bufs=1, space="PSUM"))
    ps_zct = ctx.enter_context(tc.tile_pool(name="ps_zct", bufs=2, space="PSUM"))
    ps_hr = ctx.enter_context(tc.tile_pool(name="ps_hr", bufs=2, space="PSUM"))

    hv = h.rearrange("(c g p) d -> c p g d", p=P, g=G)
    hov = hout.rearrange("(c g p) d -> c p g d", p=P, g=G)
    cov = cout.rearrange("(c g p) m -> c p g m", p=P, g=G)

    for c in range(chunks):
        h_sb = h_pool.tile([P, G, D], F32, tag="h")
        nc.sync.dma_start(out=h_sb, in_=hv[c])
        hrec_sb = out_pool.tile([P, G * D], F32, tag="hrec")
        conc_sb = out_pool.tile([P, G, M + 1], F32, tag="conc")
        for bi in range(n_batch):
            # --- transpose 4 tiles of h into ht_sb [d-part, token-free] ---
            ht_sb = ht_pool.tile([P, DC, GB * P], F32, tag="ht")
            for gl in range(GB):
                g = bi * GB + gl
                ht_ps0 = ps_ht.tile([P, 3 * P], F32, tag="htp")
                ht_ps1 = ps_ht.tile([P, 3 * P], F32, tag="htp")
                for j in range(DC):
                    tgt = ht_ps0 if j < 3 else ht_ps1
                    nc.tensor.transpose(
                        tgt[:, (j % 3) * P : (j % 3 + 1) * P],
                        h_sb[:, g, j * P : (j + 1) * P],
                        ident,
                    )
                nc.scalar.copy(
                    out=ht_sb[:, 0:3, gl * P : (gl + 1) * P],
                    in_=ht_ps0.rearrange("p (j t) -> p j t", j=3),
                )
                nc.scalar.copy(
                    out=ht_sb[:, 3:6, gl * P : (gl + 1) * P],
                    in_=ht_ps1.rearrange("p (j t) -> p j t", j=3),
                )

            # --- encoder: logits^T [64, 512] = sum_j WeT_j.T @ hT_j ---
            lgT_ps = ps_lg.tile([M, GB * P], F32, tag="lg")
            for j in range(DC):
                nc.tensor.matmul(
                    lgT_ps,
                    lhsT=wet[:, j, :],
                    rhs=ht_sb[:, j, :],
                    start=(j == 0),
                    stop=(j == DC - 1),
                )
            lgT_sb = lgt_pool.tile([M, GB * P], F32, tag="lgt")
            nc.scalar.copy(out=lgT_sb, in_=lgT_ps)

            # group tiles for the batched sparsemax tail
            s16g = grp.tile([P, GB, 16], F32, tag="s16")
            csg = grp.tile([P, GB, 16], F32, tag="cs")
            t1g = grp.tile([P, GB, 16], F32, tag="t1")
            sindg = grp.tile([P, GB, 16], F32, tag="sind")
            sml = grp.tile([P, 8, GB], F32, tag="sml")
            kg, kinvg, ssumg, taung = (sml[:, i, :] for i in range(4))
            psumg, pinvg = sml[:, 4, :], sml[:, 5, :]
            z_t, p_t = [], []

            for gl in range(GB):
                # z [128tok, 64] = transpose of logits^T slice, + be
                z_ps = ps_zct.tile([P, M], F32, tag="zct")
                nc.tensor.transpose(
                    z_ps, lgT_sb[:, gl * P : (gl + 1) * P], ident[0:M, 0:M]
                )
                z = mid.tile([P, M], F32, tag=f"z{gl}")
                nc.vector.scalar_tensor_tensor(
                    out=z, in0=z_ps, scalar=0.0, in1=be_b, op0=OP.add, op1=OP.add
                )
                z_t.append(z)

                # sorted top-16 per token
                z2 = mid.tile([P, M], F32, tag=f"z2{gl}")
                nc.vector.max(out=s16g[:, gl, 0:8], in_=z)
                nc.vector.match_replace(
                    out=z2, in_to_replace=s16g[:, gl, 0:8], in_values=z,
                    imm_value=NEG,
                )
                nc.vector.max(out=s16g[:, gl, 8:16], in_=z2)
                nc.vector.tensor_tensor_scan(
                    out=csg[:, gl, :], data0=ones16, data1=s16g[:, gl, :],
                    initial=0.0, op0=OP.mult, op1=OP.add,
                )

            # batched support/tau for the 4 tiles:
            # support = (z_sorted * rho + 1) > cumsum ; k = sum(support)
            # ssum = sum(z_sorted * support) ; tau_neg = (1 - ssum) / k
            nc.gpsimd.tensor_tensor(out=t1g, in0=s16g, in1=rho, op=OP.mult)
            nc.vector.scalar_tensor_tensor(
                out=sindg, in0=t1g, scalar=1.0, in1=csg, op0=OP.add, op1=OP.is_gt
            )
            nc.vector.reduce_sum(out=kg, in_=sindg, axis=mybir.AxisListType.X)
            nc.vector.scalar_tensor_tensor(
                out=t1g, in0=s16g, scalar=0.0, in1=sindg, op0=OP.add, op1=OP.mult
            )
            nc.vector.reduce_sum(out=ssumg, in_=t1g, axis=mybir.AxisListType.X)
            nc.vector.reciprocal(out=kinvg, in_=kg)
            nc.vector.scalar_tensor_tensor(
                out=taung, in0=ssumg, scalar=-1.0, in1=ones4,
                op0=OP.mult, op1=OP.add,
            )
            nc.vector.tensor_tensor(out=taung, in0=taung, in1=kinvg, op=OP.mult)

            for gl in range(GB):
                # p = relu(z + tau_neg); psum accumulated by the same ACT op
                p = mid.tile([P, M], F32, tag=f"p{gl}")
                nc.scalar.activation(
                    out=p, in_=z_t[gl], func=ACTF.Relu,
                    bias=taung[:, gl : gl + 1], scale=1.0,
                    accum_out=psumg[:, gl : gl + 1],
                )
                p_t.append(p)
            nc.vector.reciprocal(out=pinvg, in_=psumg)

            for gl in range(GB):
                g = bi * GB + gl
                # concepts = (z >= z_sorted[7]) * p / psum ; ones col for bias
                mask = mid.tile([P, M], F32, tag=f"mask{gl}")
                nc.vector.tensor_scalar(
                    out=mask, in0=z_t[gl], scalar1=s16g[:, gl, 7:8],
                    scalar2=None, op0=OP.is_ge,
                )
                nc.vector.scalar_tensor_tensor(
                    out=conc_sb[:, g, 0:M], in0=p_t[gl],
                    scalar=pinvg[:, gl : gl + 1], in1=mask,
                    op0=OP.mult, op1=OP.mult,
                )
                nc.gpsimd.memset(conc_sb[:, g, M : M + 1], 1.0)

                # --- decoder ---
                ct_ps = ps_zct.tile([M + 1, P], F32, tag="zct")
                nc.tensor.transpose(ct_ps, conc_sb[:, g, :], ident)
                ct_sb = ct_pool.tile([M + 1, P], F32, tag="ct")
                nc.scalar.copy(out=ct_sb, in_=ct_ps)
                hr_ps0 = ps_hr.tile([P, 384], F32, tag="hr")
                hr_ps1 = ps_hr.tile([P, 384], F32, tag="hr")
                nc.tensor.matmul(hr_ps0, lhsT=ct_sb, rhs=wdt[:, 0:384],
                                 start=True, stop=True)
                nc.tensor.matmul(hr_ps1, lhsT=ct_sb, rhs=wdt[:, 384:768],
                                 start=True, stop=True)
                nc.scalar.copy(out=hrec_sb[:, g * D : g * D + 384], in_=hr_ps0)
                nc.vector.tensor_copy(
                    out=hrec_sb[:, g * D + 384 : (g + 1) * D], in_=hr_ps1
                )

        nc.sync.dma_start(out=hov[c], in_=hrec_sb.rearrange("p (g d) -> p g d", g=G))
        nc.sync.dma_start(out=cov[c], in_=conc_sb[:, :, 0:M])


def build_nc(tok_per_core: int) -> bass.Bass:
    from contextlib import ExitStack

    nc = bacc.Bacc()
    h = nc.dram_tensor("h", [tok_per_core, D], F32, kind="ExternalInput")
    We = nc.dram_tensor("We", [M, D], F32, kind="ExternalInput")
    be = nc.dram_tensor("be", [M], F32, kind="ExternalInput")
    Wd = nc.dram_tensor("Wd", [D, M], F32, kind="ExternalInput")
    bd = nc.dram_tensor("bd", [D], F32, kind="ExternalInput")
    hout = nc.dram_tensor("h_out", [tok_per_core, D], F32, kind="ExternalOutput")
    cout = nc.dram_tensor("concepts", [tok_per_core, M], F32, kind="ExternalOutput")
    with tile.TileContext(nc) as tc:
        with ExitStack() as ctx:
            _kernel_body(
                tc, ctx, h[:], We[:], be[:], Wd[:], bd[:], hout[:], cout[:],
                tok_per_core,
            )
    nc.finalize()  # Bacc: legalize multi-sem waits (event sems), alloc regs
    return nc


_NC_CACHE: dict[int, bass.Bass] = {}
LAST_RESULTS = None  # BassKernelResults of the most recent kernel() call


def kernel(h, We, be, Wd, bd, **run_kwargs):
    global LAST_RESULTS
    h = np.ascontiguousarray(np.asarray(h, dtype=np.float32))
    We = np.ascontiguousarray(np.asarray(We, dtype=np.float32))
    be = np.ascontiguousarray(np.asarray(be, dtype=np.float32))
    Wd = np.ascontiguousarray(np.asarray(Wd, dtype=np.float32))
    bd = np.ascontiguousarray(np.asarray(bd, dtype=np.float32))

    b, s, d = h.shape
    tok = b * s
    tok_per_core = tok // N_CORES
    hf = h.reshape(tok, d)

    if tok_per_core not in _NC_CACHE:
        _NC_CACHE[tok_per_core] = build_nc(tok_per_core)
    nc = _NC_CACHE[tok_per_core]

    in_maps = [
        {
            "h": hf[i * tok_per_core : (i + 1) * tok_per_core],
            "We": We,
            "be": be,
            "Wd": Wd,
            "bd": bd,
        }
        for i in range(N_CORES)
    ]
    res = run_bass_kernel_spmd(nc, in_maps, core_ids=list(range(N_CORES)), **run_kwargs)
    LAST_RESULTS = res
    h_out = np.concatenate([r["h_out"] for r in res.results], axis=0).reshape(b, s, d)
    concepts = np.concatenate([r["concepts"] for r in res.results], axis=0).reshape(b, s, M)
    return h_out, concepts


# revision 14
# speedup vs baseline: 1.1056x; 1.1056x over previous
"""Trainium2 Bass kernel for nn_ConceptLayer (sparsemax + top-8 concept layer).

Computes, per token t (row of h):
    logits = h @ We.T + be                      # [*, 64]
    p      = sparsemax(logits)                  # entmax15() in the reference
    c      = top8_sparsify(p)                   # keep top-8 values in place
    h_out  = c @ Wd.T + bd                      # alpha = 1.0 -> h_out == h_rec

Sharding: pure data parallel over tokens (16*4096 = 65536 tokens) across 8
NeuronCores; tiny weights replicated.

Per-core dataflow (tokens tiled 128 at a time; 8 tiles per DMA chunk; compute
batched over 4-tile groups):
  - PE transposes each [128, 128] h chunk (fp32 has no DMA-transpose path);
    encoder runs with WeT stationary and 512 tokens moving per matmul,
    accumulating logits^T [64, 512] in PSUM; a small PE transpose flips each
    tile's logits back to [128tok, 64].
  - sparsemax via sorted top-16 (nc.vector.max / match_replace / max gives the
    16 largest, descending; measured max support size is 9), cumsum via
    tensor_tensor_scan, support/tau via fused scalar_tensor_tensor ops batched
    across the 4-tile group.
  - top-8 mask is `z >= z_sorted[7]` (bit-exact compare against the 8th
    largest logit -- monotone-equivalent to masking p).
  - decoder: PE-transpose concepts (augmented with a ones column so the
    matmul adds bd via an extra contraction row), 2 matmuls of N=384.
"""

import os
import sys

import numpy as np

for _p in ("/opt/trn_rl_repo", "/root/.axon_site/_ro/trn_rl_repo"):
    if os.path.isdir(_p) and _p not in sys.path:
        sys.path.insert(0, _p)

import concourse.bass as bass
import concourse.bacc as bacc
import concourse.tile as tile
from concourse import mybir
from concourse.bass_utils import run_bass_kernel_spmd
from concourse.masks import make_identity

F32 = mybir.dt.float32
OP = mybir.AluOpType
ACTF = mybir.ActivationFunctionType

N_CORES = 8
B, S, D, M = 16, 4096, 768, 64
TOK = B * S
P = 128
DC = D // P          # 6 d-chunks of 128
GB = 4               # token-tiles per compute batch (512 moving columns)
NEG = -1.0e30


def _broadcast_ap(src: bass.AP, parts: int) -> bass.AP:
    """View a 1-D DRAM tensor as [parts, n] with 0-stride partition dim."""
    return bass.AP(tensor=src.tensor, offset=src.offset, ap=[[0, parts]] + list(src.ap))


def _kernel_body(tc: tile.TileContext, ctx, h, We, be, Wd, bd, hout, cout, tok_per_core):
    nc = tc.nc
    tiles = tok_per_core // P
    G = 8 if tiles % 8 == 0 else GB   # token-tiles per DMA chunk
    chunks = tiles // G
    n_batch = G // GB                 # compute batches per chunk

    consts = ctx.enter_context(tc.tile_pool(name="consts", bufs=1))

    ident = consts.tile([P, P], F32)
    make_identity(nc, ident)

    # rho64[p, g, j] = j+1 (j in 0..15) for the batched support test; ones for
    # the cumsum scan / tau.
    rho_i = consts.tile([P, GB, 16], mybir.dt.int32)
    nc.gpsimd.iota(rho_i, pattern=[[0, GB], [1, 16]], base=1, channel_multiplier=0)
    rho = consts.tile([P, GB, 16], F32)
    nc.vector.tensor_copy(rho, rho_i)
    ones16 = consts.tile([P, 16], F32)
    nc.vector.memset(ones16, 1.0)
    ones4 = ones16[:, 0:GB]

    # be broadcast to [P, M] (added to logits during the PSUM->SBUF copy).
    be_b = consts.tile([P, M], F32)
    nc.gpsimd.dma_start(out=be_b, in_=_broadcast_ap(be, P))

    # Encoder weights: We [M, D] -> WeT chunks [P, DC, M] (WeT[d, c] = We[c, d]).
    # Decoder weights: Wd [D, M] -> WdT_aug [M+1, D]; row M = bd so the ones
    # column of the augmented concepts adds the bias.
    we_sb = consts.tile([M, D], F32)
    nc.sync.dma_start(out=we_sb, in_=We)
    wet = consts.tile([P, DC, M], F32)
    wd_view = Wd.rearrange("(j p) m -> p j m", p=P)
    wd_sb = consts.tile([P, DC, M], F32)
    nc.sync.dma_start(out=wd_sb, in_=wd_view)
    wdt = consts.tile([M + 1, D], F32)
    with tc.tile_pool(name="ps_setup", bufs=2, space="PSUM") as ps_setup:
        for j in range(DC):
            pt = ps_setup.tile([P, M], F32, tag="setup")
            nc.tensor.transpose(pt, we_sb[:, j * P : (j + 1) * P], ident[0:M, 0:M])
            nc.scalar.copy(out=wet[:, j, :], in_=pt)
        for j in range(DC):
            pt = ps_setup.tile([M, P], F32, tag="setup")
            nc.tensor.transpose(pt, wd_sb[:, j, :], ident)
            nc.scalar.copy(out=wdt[0:M, j * P : (j + 1) * P], in_=pt)
    nc.gpsimd.dma_start(out=wdt[M : M + 1, :], in_=_broadcast_ap(bd, 1))

    # Streaming pools.
    h_pool = ctx.enter_context(tc.tile_pool(name="h_in", bufs=3))
    ht_pool = ctx.enter_context(tc.tile_pool(name="ht", bufs=3))
    mid = ctx.enter_context(tc.tile_pool(name="mid", bufs=2))
    grp = ctx.enter_context(tc.tile_pool(name="grp", bufs=2))
    ct_pool = ctx.enter_context(tc.tile_pool(name="ct", bufs=2))
    out_pool = ctx.enter_context(tc.tile_pool(name="out", bufs=2))
    ps_ht = ctx.enter_context(tc.tile_pool(name="ps_ht", bufs=3, space="PSUM"))
    ps_lg = ctx.enter_context(tc.tile_pool(name="ps_lg", bufs=2, space="PSUM"))
    ps_ct = ctx.enter_context(tc.tile_pool(name="ps_ct", bufs=1, space="PSUM"))
    ps_hr = ctx.enter_context(tc.tile_pool(name="ps_hr", bufs=2, space="PSUM"))

    hv = h.rearrange("(c g p) d -> c p g d", p=P, g=G)
    hov = hout.rearrange("(c g p) d -> c p g d", p=P, g=G)
    cov = cout.rearrange("(c g p) m -> c p g m", p=P, g=G)

    for c in range(chunks):
        h_sb = h_pool.tile([P, G, D], F32, tag="h")
        nc.sync.dma_start(out=h_sb, in_=hv[c])
        hrec_sb = out_pool.tile([P, G * D], F32, tag="hrec")
        conc_sb = out_pool.tile([P, G, M + 1], F32, tag="conc")
        for bi in range(n_batch):
            # group tiles for the batched sparsemax tail
            s16g = grp.tile([P, GB, 16], F32, tag="s16")
            csg = grp.tile([P, GB, 16], F32, tag="cs")
            t1g = grp.tile([P, GB, 16], F32, tag="t1")
            sindg = grp.tile([P, GB, 16], F32, tag="sind")
            sml = grp.tile([P, 8, GB], F32, tag="sml")
            kg, kinvg, ssumg, taung = (sml[:, i, :] for i in range(4))
            psumg, pinvg = sml[:, 4, :], sml[:, 5, :]
            z_t, p_t = [], []

            for gl in range(GB):
                g = bi * GB + gl
                # --- encoder: transpose h tile, matmul against WeT ---
                ht_ps0 = ps_ht.tile([P, 3 * P], F32, tag="htp")
                ht_ps1 = ps_ht.tile([P, 3 * P], F32, tag="htp")
                for j in range(DC):
                    tgt = ht_ps0 if j < 3 else ht_ps1
                    nc.tensor.transpose(
                        tgt[:, (j % 3) * P : (j % 3 + 1) * P],
                        h_sb[:, g, j * P : (j + 1) * P],
                        ident,
                    )
                ht_sb = ht_pool.tile([P, DC * P], F32, tag="ht")
                nc.scalar.copy(out=ht_sb[:, 0 : 3 * P], in_=ht_ps0)
                nc.scalar.copy(out=ht_sb[:, 3 * P : 6 * P], in_=ht_ps1)

                lg_ps = ps_lg.tile([P, M], F32, tag="lg")
                for j in range(DC):
                    nc.tensor.matmul(
                        lg_ps,
                        lhsT=ht_sb[:, j * P : (j + 1) * P],
                        rhs=wet[:, j, :],
                        start=(j == 0),
                        stop=(j == DC - 1),
                    )

                # z = logits + be  (PSUM -> SBUF)
                z = mid.tile([P, M], F32, tag=f"z{gl}")
                nc.vector.scalar_tensor_tensor(
                    out=z, in0=lg_ps, scalar=0.0, in1=be_b, op0=OP.add, op1=OP.add
                )
                z_t.append(z)

                # sorted top-16 per token
                z2 = mid.tile([P, M], F32, tag=f"z2{gl}")
                nc.vector.max(out=s16g[:, gl, 0:8], in_=z)
                nc.vector.match_replace(
                    out=z2, in_to_replace=s16g[:, gl, 0:8], in_values=z,
                    imm_value=NEG,
                )
                nc.vector.max(out=s16g[:, gl, 8:16], in_=z2)
                nc.vector.tensor_tensor_scan(
                    out=csg[:, gl, :], data0=ones16, data1=s16g[:, gl, :],
                    initial=0.0, op0=OP.mult, op1=OP.add,
                )

            # batched support/tau for the 4 tiles:
            # support = (z_sorted * rho + 1) > cumsum ; k = sum(support)
            # ssum = sum(z_sorted * support) ; tau_neg = (1 - ssum) / k
            nc.gpsimd.tensor_tensor(out=t1g, in0=s16g, in1=rho, op=OP.mult)
            nc.vector.scalar_tensor_tensor(
                out=sindg, in0=t1g, scalar=1.0, in1=csg, op0=OP.add, op1=OP.is_gt
            )
            nc.vector.reduce_sum(out=kg, in_=sindg, axis=mybir.AxisListType.X)
            nc.vector.scalar_tensor_tensor(
                out=t1g, in0=s16g, scalar=0.0, in1=sindg, op0=OP.add, op1=OP.mult
            )
            nc.vector.reduce_sum(out=ssumg, in_=t1g, axis=mybir.AxisListType.X)
            nc.vector.reciprocal(out=kinvg, in_=kg)
            nc.vector.scalar_tensor_tensor(
                out=taung, in0=ssumg, scalar=-1.0, in1=ones4,
                op0=OP.mult, op1=OP.add,
            )
            nc.vector.tensor_tensor(out=taung, in0=taung, in1=kinvg, op=OP.mult)

            for gl in range(GB):
                # p = relu(z + tau_neg); psum accumulated by the same ACT op
                p = mid.tile([P, M], F32, tag=f"p{gl}")
                nc.scalar.activation(
                    out=p, in_=z_t[gl], func=ACTF.Relu,
                    bias=taung[:, gl : gl + 1], scale=1.0,
                    accum_out=psumg[:, gl : gl + 1],
                )
                p_t.append(p)
            nc.vector.reciprocal(out=pinvg, in_=psumg)

            for gl in range(GB):
                g = bi * GB + gl
                # concepts = (z >= z_sorted[7]) * p / psum ; ones col for bias
                mask = mid.tile([P, M], F32, tag=f"mask{gl}")
                nc.vector.tensor_scalar(
                    out=mask, in0=z_t[gl], scalar1=s16g[:, gl, 7:8],
                    scalar2=None, op0=OP.is_ge,
                )
                nc.vector.scalar_tensor_tensor(
                    out=conc_sb[:, g, 0:M], in0=p_t[gl],
                    scalar=pinvg[:, gl : gl + 1], in1=mask,
                    op0=OP.mult, op1=OP.mult,
                )
                nc.gpsimd.memset(conc_sb[:, g, M : M + 1], 1.0)

                # --- decoder ---
                ct_ps = ps_ct.tile([M + 1, P], F32, tag="ct")
                nc.tensor.transpose(ct_ps, conc_sb[:, g, :], ident)
                ct_sb = ct_pool.tile([M + 1, P], F32, tag="ct")
                nc.scalar.copy(out=ct_sb, in_=ct_ps)
                hr_ps0 = ps_hr.tile([P, 384], F32, tag="hr")
                hr_ps1 = ps_hr.tile([P, 384], F32, tag="hr")
                nc.tensor.matmul(hr_ps0, lhsT=ct_sb, rhs=wdt[:, 0:384],
                                 start=True, stop=True)
                nc.tensor.matmul(hr_ps1, lhsT=ct_sb, rhs=wdt[:, 384:768],
                                 start=True, stop=True)
                nc.scalar.copy(out=hrec_sb[:, g * D : g * D + 384], in_=hr_ps0)
                nc.vector.tensor_copy(
                    out=hrec_sb[:, g * D + 384 : (g + 1) * D], in_=hr_ps1
                )

        nc.sync.dma_start(out=hov[c], in_=hrec_sb.rearrange("p (g d) -> p g d", g=G))
        nc.sync.dma_start(out=cov[c], in_=conc_sb[:, :, 0:M])


def build_nc(tok_per_core: int) -> bass.Bass:
    from contextlib import ExitStack

    nc = bacc.Bacc()
    h = nc.dram_tensor("h", [tok_per_core, D], F32, kind="ExternalInput")
    We = nc.dram_tensor("We", [M, D], F32, kind="ExternalInput")
    be = nc.dram_tensor("be", [M], F32, kind="ExternalInput")
    Wd = nc.dram_tensor("Wd", [D, M], F32, kind="ExternalInput")
    bd = nc.dram_tensor("bd", [D], F32, kind="ExternalInput")
    hout = nc.dram_tensor("h_out", [tok_per_core, D], F32, kind="ExternalOutput")
    cout = nc.dram_tensor("concepts", [tok_per_core, M], F32, kind="ExternalOutput")
    with tile.TileContext(nc) as tc:
        with ExitStack() as ctx:
            _kernel_body(
                tc, ctx, h[:], We[:], be[:], Wd[:], bd[:], hout[:], cout[:],
                tok_per_core,
            )
    nc.finalize()  # Bacc: legalize multi-sem waits (event sems), alloc regs
    return nc


_NC_CACHE: dict[int, bass.Bass] = {}
LAST_RESULTS = None  # BassKernelResults of the most recent kernel() call


def kernel(h, We, be, Wd, bd, **run_kwargs):
    global LAST_RESULTS
    h = np.ascontiguousarray(np.asarray(h, dtype=np.float32))
    We = np.ascontiguousarray(np.asarray(We, dtype=np.float32))
    be = np.ascontiguousarray(np.asarray(be, dtype=np.float32))
    Wd = np.ascontiguousarray(np.asarray(Wd, dtype=np.float32))
    bd = np.ascontiguousarray(np.asarray(bd, dtype=np.float32))

    b, s, d = h.shape
    tok = b * s
    tok_per_core = tok // N_CORES
    hf = h.reshape(tok, d)

    if tok_per_core not in _NC_CACHE:
        _NC_CACHE[tok_per_core] = build_nc(tok_per_core)
    nc = _NC_CACHE[tok_per_core]

    in_maps = [
        {
            "h": hf[i * tok_per_core : (i + 1) * tok_per_core],
            "We": We,
            "be": be,
            "Wd": Wd,
            "bd": bd,
        }
        for i in range(N_CORES)
    ]
    res = run_bass_kernel_spmd(nc, in_maps, core_ids=list(range(N_CORES)), **run_kwargs)
    LAST_RESULTS = res
    h_out = np.concatenate([r["h_out"] for r in res.results], axis=0).reshape(b, s, d)
    concepts = np.concatenate([r["concepts"] for r in res.results], axis=0).reshape(b, s, M)
    return h_out, concepts


# revision 15
# speedup vs baseline: 1.1896x; 1.0760x over previous
"""Trainium2 Bass kernel for nn_ConceptLayer (sparsemax + top-8 concept layer).

Computes, per token t (row of h):
    logits = h @ We.T + be                      # [*, 64]
    p      = sparsemax(logits)                  # entmax15() in the reference
    c      = top8_sparsify(p)                   # keep top-8 values in place
    h_out  = c @ Wd.T + bd                      # alpha = 1.0 -> h_out == h_rec

Sharding: pure data parallel over tokens (16*4096 = 65536 tokens) across 8
NeuronCores; tiny weights replicated.

Per-core dataflow (tokens tiled 128 at a time; 8 tiles per DMA chunk; compute
batched over 4-tile groups):
  - PE transposes each [128, 128] h chunk (fp32 has no DMA-transpose path);
    encoder runs with WeT stationary and 512 tokens moving per matmul,
    accumulating logits^T [64, 512] in PSUM; a small PE transpose flips each
    tile's logits back to [128tok, 64].
  - sparsemax via sorted top-16 (nc.vector.max / match_replace / max gives the
    16 largest, descending; measured max support size is 9), cumsum via
    tensor_tensor_scan, support/tau via fused scalar_tensor_tensor ops batched
    across the 4-tile group.
  - top-8 mask is `z >= z_sorted[7]` (bit-exact compare against the 8th
    largest logit -- monotone-equivalent to masking p).
  - decoder: PE-transpose concepts (augmented with a ones column so the
    matmul adds bd via an extra contraction row), 2 matmuls of N=384.
"""

import os
import sys

import numpy as np

for _p in ("/opt/trn_rl_repo", "/root/.axon_site/_ro/trn_rl_repo"):
    if os.path.isdir(_p) and _p not in sys.path:
        sys.path.insert(0, _p)

import concourse.bass as bass
import concourse.bacc as bacc
import concourse.tile as tile
from concourse import mybir
from concourse.bass_utils import run_bass_kernel_spmd
from concourse.masks import make_identity

F32 = mybir.dt.float32
OP = mybir.AluOpType
ACTF = mybir.ActivationFunctionType

N_CORES = 8
B, S, D, M = 16, 4096, 768, 64
TOK = B * S
P = 128
DC = D // P          # 6 d-chunks of 128
GB = 4               # token-tiles per compute batch (512 moving columns)
NEG = -1.0e30


def _broadcast_ap(src: bass.AP, parts: int) -> bass.AP:
    """View a 1-D DRAM tensor as [parts, n] with 0-stride partition dim."""
    return bass.AP(tensor=src.tensor, offset=src.offset, ap=[[0, parts]] + list(src.ap))


def _kernel_body(tc: tile.TileContext, ctx, h, We, be, Wd, bd, hout, cout, tok_per_core):
    nc = tc.nc
    tiles = tok_per_core // P
    G = 8 if tiles % 8 == 0 else GB   # token-tiles per DMA chunk
    chunks = tiles // G
    n_batch = G // GB                 # compute batches per chunk

    consts = ctx.enter_context(tc.tile_pool(name="consts", bufs=1))

    ident = consts.tile([P, P], F32)
    make_identity(nc, ident)

    # rho64[p, g, j] = j+1 (j in 0..15) for the batched support test; ones for
    # the cumsum scan / tau.
    rho_i = consts.tile([P, GB, 16], mybir.dt.int32)
    nc.gpsimd.iota(rho_i, pattern=[[0, GB], [1, 16]], base=1, channel_multiplier=0)
    rho = consts.tile([P, GB, 16], F32)
    nc.vector.tensor_copy(rho, rho_i)
    ones16 = consts.tile([P, 16], F32)
    nc.vector.memset(ones16, 1.0)
    ones4 = ones16[:, 0:GB]

    # be broadcast to [P, M] (added to logits during the PSUM->SBUF copy).
    be_b = consts.tile([P, M], F32)
    nc.gpsimd.dma_start(out=be_b, in_=_broadcast_ap(be, P))

    # Encoder weights: We [M, D] -> WeT chunks [P, DC, M] (WeT[d, c] = We[c, d]).
    # Decoder weights: Wd [D, M] -> WdT_aug [M+1, D]; row M = bd so the ones
    # column of the augmented concepts adds the bias.
    we_sb = consts.tile([M, D], F32)
    nc.sync.dma_start(out=we_sb, in_=We)
    wet = consts.tile([P, DC, M], F32)
    wd_view = Wd.rearrange("(j p) m -> p j m", p=P)
    wd_sb = consts.tile([P, DC, M], F32)
    nc.sync.dma_start(out=wd_sb, in_=wd_view)
    wdt = consts.tile([M + 1, D], F32)
    with tc.tile_pool(name="ps_setup", bufs=2, space="PSUM") as ps_setup:
        for j in range(DC):
            pt = ps_setup.tile([P, M], F32, tag="setup")
            nc.tensor.transpose(pt, we_sb[:, j * P : (j + 1) * P], ident[0:M, 0:M])
            nc.scalar.copy(out=wet[:, j, :], in_=pt)
        for j in range(DC):
            pt = ps_setup.tile([M, P], F32, tag="setup")
            nc.tensor.transpose(pt, wd_sb[:, j, :], ident)
            nc.scalar.copy(out=wdt[0:M, j * P : (j + 1) * P], in_=pt)
    nc.gpsimd.dma_start(out=wdt[M : M + 1, :], in_=_broadcast_ap(bd, 1))

    # Streaming pools.
    h_pool = ctx.enter_context(tc.tile_pool(name="h_in", bufs=3))
    ht_pool = ctx.enter_context(tc.tile_pool(name="ht", bufs=3))
    mid = ctx.enter_context(tc.tile_pool(name="mid", bufs=2))
    grp = ctx.enter_context(tc.tile_pool(name="grp", bufs=2))
    ct_pool = ctx.enter_context(tc.tile_pool(name="ct", bufs=2))
    out_pool = ctx.enter_context(tc.tile_pool(name="out", bufs=2))
    ps_ht = ctx.enter_context(tc.tile_pool(name="ps_ht", bufs=3, space="PSUM"))
    ps_lg = ctx.enter_context(tc.tile_pool(name="ps_lg", bufs=2, space="PSUM"))
    ps_ct = ctx.enter_context(tc.tile_pool(name="ps_ct", bufs=1, space="PSUM"))
    ps_hr = ctx.enter_context(tc.tile_pool(name="ps_hr", bufs=2, space="PSUM"))

    hv = h.rearrange("(c g p) d -> c p g d", p=P, g=G)
    hov = hout.rearrange("(c g p) d -> c p g d", p=P, g=G)
    cov = cout.rearrange("(c g p) m -> c p g m", p=P, g=G)

    for c in range(chunks):
        h_sb = h_pool.tile([P, G, D], F32, tag="h")
        for bi in range(n_batch):
            nc.sync.dma_start(
                out=h_sb[:, bi * GB : (bi + 1) * GB, :],
                in_=hv[c, :, bi * GB : (bi + 1) * GB, :],
            )
        hrec_sb = out_pool.tile([P, G * D], F32, tag="hrec")
        conc_sb = out_pool.tile([P, G, M + 1], F32, tag="conc")
        for bi in range(n_batch):
            # group tiles for the batched sparsemax tail
            s16g = grp.tile([P, GB, 16], F32, tag="s16")
            csg = grp.tile([P, GB, 16], F32, tag="cs")
            t1g = grp.tile([P, GB, 16], F32, tag="t1")
            sindg = grp.tile([P, GB, 16], F32, tag="sind")
            sml = grp.tile([P, 8, GB], F32, tag="sml")
            kg, kinvg, ssumg, taung = (sml[:, i, :] for i in range(4))
            psumg, pinvg = sml[:, 4, :], sml[:, 5, :]
            z_t, p_t = [], []

            for gl in range(GB):
                g = bi * GB + gl
                # --- encoder: transpose h tile, matmul against WeT ---
                ht_ps0 = ps_ht.tile([P, 3 * P], F32, tag="htp")
                ht_ps1 = ps_ht.tile([P, 3 * P], F32, tag="htp")
                for j in range(DC):
                    tgt = ht_ps0 if j < 3 else ht_ps1
                    nc.tensor.transpose(
                        tgt[:, (j % 3) * P : (j % 3 + 1) * P],
                        h_sb[:, g, j * P : (j + 1) * P],
                        ident,
                    )
                ht_sb = ht_pool.tile([P, DC * P], F32, tag="ht")
                nc.scalar.copy(out=ht_sb[:, 0 : 3 * P], in_=ht_ps0)
                nc.scalar.copy(out=ht_sb[:, 3 * P : 6 * P], in_=ht_ps1)

                lg_ps = ps_lg.tile([P, M], F32, tag="lg")
                for j in range(DC):
                    nc.tensor.matmul(
                        lg_ps,
                        lhsT=ht_sb[:, j * P : (j + 1) * P],
                        rhs=wet[:, j, :],
                        start=(j == 0),
                        stop=(j == DC - 1),
                    )

                # z = logits + be  (PSUM -> SBUF)
                z = mid.tile([P, M], F32, tag=f"z{gl}")
                nc.vector.scalar_tensor_tensor(
                    out=z, in0=lg_ps, scalar=0.0, in1=be_b, op0=OP.add, op1=OP.add
                )
                z_t.append(z)

                # sorted top-16 per token
                z2 = mid.tile([P, M], F32, tag=f"z2{gl}")
                nc.vector.max(out=s16g[:, gl, 0:8], in_=z)
                nc.vector.match_replace(
                    out=z2, in_to_replace=s16g[:, gl, 0:8], in_values=z,
                    imm_value=NEG,
                )
                nc.vector.max(out=s16g[:, gl, 8:16], in_=z2)
                nc.vector.tensor_tensor_scan(
                    out=csg[:, gl, :], data0=ones16, data1=s16g[:, gl, :],
                    initial=0.0, op0=OP.mult, op1=OP.add,
                )

            # batched support/tau for the 4 tiles:
            # support = (z_sorted * rho + 1) > cumsum ; k = sum(support)
            # ssum = sum(z_sorted * support) ; tau_neg = (1 - ssum) / k
            nc.gpsimd.tensor_tensor(out=t1g, in0=s16g, in1=rho, op=OP.mult)
            nc.vector.scalar_tensor_tensor(
                out=sindg, in0=t1g, scalar=1.0, in1=csg, op0=OP.add, op1=OP.is_gt
            )
            nc.vector.reduce_sum(out=kg, in_=sindg, axis=mybir.AxisListType.X)
            nc.vector.scalar_tensor_tensor(
                out=t1g, in0=s16g, scalar=0.0, in1=sindg, op0=OP.add, op1=OP.mult
            )
            nc.vector.reduce_sum(out=ssumg, in_=t1g, axis=mybir.AxisListType.X)
            nc.vector.reciprocal(out=kinvg, in_=kg)
            nc.vector.scalar_tensor_tensor(
                out=taung, in0=ssumg, scalar=-1.0, in1=ones4,
                op0=OP.mult, op1=OP.add,
            )
            nc.vector.tensor_tensor(out=taung, in0=taung, in1=kinvg, op=OP.mult)

            for gl in range(GB):
                # p = relu(z + tau_neg); psum accumulated by the same ACT op
                p = mid.tile([P, M], F32, tag=f"p{gl}")
                nc.scalar.activation(
                    out=p, in_=z_t[gl], func=ACTF.Relu,
                    bias=taung[:, gl : gl + 1], scale=1.0,
                    accum_out=psumg[:, gl : gl + 1],
                )
                p_t.append(p)
            nc.vector.reciprocal(out=pinvg, in_=psumg)

            for gl in range(GB):
                g = bi * GB + gl
                # concepts = (z >= z_sorted[7]) * p / psum ; ones col for bias
                mask = mid.tile([P, M], F32, tag=f"mask{gl}")
                nc.vector.tensor_scalar(
                    out=mask, in0=z_t[gl], scalar1=s16g[:, gl, 7:8],
                    scalar2=None, op0=OP.is_ge,
                )
                nc.vector.scalar_tensor_tensor(
                    out=conc_sb[:, g, 0:M], in0=p_t[gl],
                    scalar=pinvg[:, gl : gl + 1], in1=mask,
                    op0=OP.mult, op1=OP.mult,
                )
                nc.gpsimd.memset(conc_sb[:, g, M : M + 1], 1.0)

                # --- decoder ---
                ct_ps = ps_ct.tile([M + 1, P], F32, tag="ct")
                nc.tensor.transpose(ct_ps, conc_sb[:, g, :], ident)
                ct_sb = ct_pool.tile([M + 1, P], F32, tag="ct")
                nc.scalar.copy(out=ct_sb, in_=ct_ps)
                hr_ps0 = ps_hr.tile([P, 384], F32, tag="hr")
                hr_ps1 = ps_hr.tile([P, 384], F32, tag="hr")
                nc.tensor.matmul(hr_ps0, lhsT=ct_sb, rhs=wdt[:, 0:384],
                                 start=True, stop=True)
                nc.tensor.matmul(hr_ps1, lhsT=ct_sb, rhs=wdt[:, 384:768],
                                 start=True, stop=True)
                nc.scalar.copy(out=hrec_sb[:, g * D : g * D + 384], in_=hr_ps0)
                nc.vector.tensor_copy(
                    out=hrec_sb[:, g * D + 384 : (g + 1) * D], in_=hr_ps1
                )

            nc.sync.dma_start(
                out=hov[c, :, bi * GB : (bi + 1) * GB, :],
                in_=hrec_sb.rearrange("p (g d) -> p g d", g=G)[
                    :, bi * GB : (bi + 1) * GB, :
                ],
            )
        nc.sync.dma_start(out=cov[c], in_=conc_sb[:, :, 0:M])


def build_nc(tok_per_core: int) -> bass.Bass:
    from contextlib import ExitStack

    nc = bacc.Bacc()
    h = nc.dram_tensor("h", [tok_per_core, D], F32, kind="ExternalInput")
    We = nc.dram_tensor("We", [M, D], F32, kind="ExternalInput")
    be = nc.dram_tensor("be", [M], F32, kind="ExternalInput")
    Wd = nc.dram_tensor("Wd", [D, M], F32, kind="ExternalInput")
    bd = nc.dram_tensor("bd", [D], F32, kind="ExternalInput")
    hout = nc.dram_tensor("h_out", [tok_per_core, D], F32, kind="ExternalOutput")
    cout = nc.dram_tensor("concepts", [tok_per_core, M], F32, kind="ExternalOutput")
    with tile.TileContext(nc) as tc:
        with ExitStack() as ctx:
            _kernel_body(
                tc, ctx, h[:], We[:], be[:], Wd[:], bd[:], hout[:], cout[:],
                tok_per_core,
            )
    nc.finalize()  # Bacc: legalize multi-sem waits (event sems), alloc regs
    return nc


_NC_CACHE: dict[int, bass.Bass] = {}
LAST_RESULTS = None  # BassKernelResults of the most recent kernel() call


def kernel(h, We, be, Wd, bd, **run_kwargs):
    global LAST_RESULTS
    h = np.ascontiguousarray(np.asarray(h, dtype=np.float32))
    We = np.ascontiguousarray(np.asarray(We, dtype=np.float32))
    be = np.ascontiguousarray(np.asarray(be, dtype=np.float32))
    Wd = np.ascontiguousarray(np.asarray(Wd, dtype=np.float32))
    bd = np.ascontiguousarray(np.asarray(bd, dtype=np.float32))

    b, s, d = h.shape
    tok = b * s
    tok_per_core = tok // N_CORES
    hf = h.reshape(tok, d)

    if tok_per_core not in _NC_CACHE:
        _NC_CACHE[tok_per_core] = build_nc(tok_per_core)
    nc = _NC_CACHE[tok_per_core]

    in_maps = [
        {
            "h": hf[i * tok_per_core : (i + 1) * tok_per_core],
            "We": We,
            "be": be,
            "Wd": Wd,
            "bd": bd,
        }
        for i in range(N_CORES)
    ]
    res = run_bass_kernel_spmd(nc, in_maps, core_ids=list(range(N_CORES)), **run_kwargs)
    LAST_RESULTS = res
    h_out = np.concatenate([r["h_out"] for r in res.results], axis=0).reshape(b, s, d)
    concepts = np.concatenate([r["concepts"] for r in res.results], axis=0).reshape(b, s, M)
    return h_out, concepts
